# revision 31
# baseline (speedup 1.0000x reference)
"""DCVQ quantizer (vq_codebook) on 8 TRN2 NeuronCores.

Strategy (per spec sharding hint): data-parallel over tokens (B*H*W),
codebooks replicated on every core. Per core:
  - distances via TensorE matmuls (float32r, full rate):
    m[t, c] = z_t . c_c - 0.5*||c_c||^2  (the -0.5*c2 folded in via an
    appended ones-row on the stationary side); argmin d2 == argmax m.
  - argmax via DVE max8 + max_index over the [128, 1024] PSUM tile.
  - codes gathered on-device via indirect DMA from the codebook in DRAM.
  - losses: loss_vq == loss_commit == mean(min d2) from sum(z^2)
    (ScalarE square+accum) and sum(max m) (ones-matmul partition
    reduction); per-core partial sums combined on host.

kernel(z, codebooks) takes full inputs, returns
(out[B,D,H,W] f32, loss_vq f32, loss_commit f32, indices[T,N] int32)
matching reference.reference().
"""

import numpy as np


def _rep_range(repeat):
    # repeat the whole per-codebook pipeline (benchmarking aid; repeat=1
    # for normal runs)
    for _ in range(repeat):
        yield from range(N)


# ---- problem constants (hardcoded per harness rules) ----
B, D, H, W = 16, 512, 32, 32
N, M, DS = 8, 1024, 64
NCORES = 8
T = B * H * W                      # 16384 tokens
TL = T // NCORES                   # 2048 tokens per core
NT = TL // 128                     # 16 token tiles of 128
MH = M // 2                        # 512 (half the codebook)
USE_F32R = False                   # f32r flips ~0.07% of argmins (too inexact)


def _build_graph(repeat=1, variant="full"):
    """variant: 'mm' | 'max' | 'maxidx' | 'nogather' | 'full' —
    progressively larger subsets of the pipeline (benchmarking aid)."""
    import concourse.bacc as bacc
    import concourse.bass as bass
    import concourse.mybir as mybir
    from concourse.tile import TileContext

    lvl = ["mm", "max", "maxidx", "nogather", "full"].index(variant)

    fp32 = mybir.dt.float32
    u32 = mybir.dt.uint32
    mm_dt = mybir.dt.float32r if USE_F32R else fp32

    nc = bacc.Bacc("TRN2", target_bir_lowering=False, debug=False)

    zta = nc.declare_dram_parameter("zta", [N, DS + 1, TL], mm_dt, isOutput=False)
    cbta = nc.declare_dram_parameter("cbta", [N, DS + 1, M], mm_dt, isOutput=False)
    cbf = [
        nc.declare_dram_parameter(f"cbf{n}", [M, DS], fp32, isOutput=False)
        for n in range(N)
    ]
    zq = nc.declare_dram_parameter("zq", [N, NT, 128, DS], fp32, isOutput=True)
    idxp = nc.declare_dram_parameter("idxp", [128, N * NT * 8], u32, isOutput=True)
    lossp = nc.declare_dram_parameter("lossp", [1, 2], fp32, isOutput=True)

    with TileContext(nc) as tc:
        with (
            tc.tile_pool(name="cbp", bufs=2) as cbp,
            tc.tile_pool(name="zp", bufs=2) as zp,
            tc.tile_pool(name="ps", bufs=4, space="PSUM") as psp,
            tc.tile_pool(name="zqp", bufs=4) as zqp,
            tc.tile_pool(name="persist", bufs=1) as pp,
            tc.tile_pool(name="sq", bufs=2) as sqp,
            tc.tile_pool(name="fin", bufs=1) as fin,
        ):
            ix_all = pp.tile([128, N * NT * 8], u32)
            mx_all = pp.tile([128, N * NT * 8], fp32)
            zsq = pp.tile([128, N], fp32)
            nc.vector.memset(zsq[:], 0.0)
            if lvl < 4:  # benchmark variants leave parts unwritten
                nc.vector.memset(ix_all[:], 0)
                nc.vector.memset(mx_all[:], 0.0)

            for n in _rep_range(repeat):
                # DMA into raw staging tiles, then same-dtype DVE copies.
                # Matmuls read only DVE-produced tiles + PSUM guarded by DVE
                # readers, so each (self-loading, 1-wait-limited) matmul
                # carries exactly one semaphore wait (the DVE sem).
                cbt_raw = cbp.tile([DS + 1, M], mm_dt, tag="cbraw")
                nc.sync.dma_start(cbt_raw[:], cbta[n])
                zt_raw = zp.tile([DS + 1, TL], mm_dt, tag="zraw")
                nc.sync.dma_start(zt_raw[:], zta[n])
                cbt = cbp.tile([DS + 1, M], mm_dt, tag="cb")
                nc.vector.tensor_copy(cbt[:], cbt_raw[:])
                zt = zp.tile([DS + 1, TL], mm_dt, tag="z")
                nc.vector.tensor_copy(zt[:], zt_raw[:])

                # sum of z^2 for this codebook slab (loss term)
                junk = sqp.tile([DS, TL], fp32)
                nc.scalar.activation(
                    junk[:],
                    zt_raw[0:DS, :].bitcast(fp32),
                    mybir.ActivationFunctionType.Square,
                    accum_out=zsq[0:DS, n : n + 1],
                )

                for t in range(NT):
                    k = n * NT + t
                    lhs = zt[:, t * 128 : (t + 1) * 128]
                    dm = psp.tile([128, M], fp32)
                    nc.tensor.matmul(
                        dm[:, 0:MH], lhsT=lhs, rhs=cbt[:, 0:MH],
                        start=True, stop=True,
                    )
                    nc.tensor.matmul(
                        dm[:, MH:M], lhsT=lhs, rhs=cbt[:, MH:M],
                        start=True, stop=True,
                    )
                    # row max (top-8) + index of the max, straight off PSUM
                    if lvl >= 1:
                        nc.vector.max(
                            out=mx_all[:, k * 8 : (k + 1) * 8], in_=dm[:]
                        )
                    if lvl >= 2:
                        nc.vector.max_index(
                            out=ix_all[:, k * 8 : (k + 1) * 8],
                            in_max=mx_all[:, k * 8 : (k + 1) * 8],
                            in_values=dm[:],
                        )
                    if lvl >= 4:
                        # gather the winning codes
                        zg = zqp.tile([128, DS], fp32)
                        nc.gpsimd.indirect_dma_start(
                            out=zg[:],
                            out_offset=None,
                            in_=cbf[n][:],
                            in_offset=bass.IndirectOffsetOnAxis(
                                ap=ix_all[:, k * 8 : k * 8 + 1], axis=0
                            ),
                        )
                        nc.sync.dma_start(zq[n, t], zg[:])

            # ---- loss partials (partition reduction on GPSIMD) ----
            from concourse.bass_isa import ReduceOp

            mx0 = mx_all.rearrange("p (k e) -> p k e", e=8)[:, :, 0]
            mxc = fin.tile([128, N * NT], fp32, tag="mxc")
            nc.vector.tensor_copy(mxc[:], mx0)
            nc.gpsimd.partition_all_reduce(
                mxc[:], mxc[:], 128, ReduceOp.add
            )
            mxs = fin.tile([1, 1], fp32, tag="mxs")
            nc.vector.reduce_sum(
                mxs[:], mxc[0:1, :], axis=mybir.AxisListType.X
            )
            nc.gpsimd.partition_all_reduce(zsq[:], zsq[:], 128, ReduceOp.add)
            z2s = fin.tile([1, 1], fp32, tag="z2s")
            nc.vector.reduce_sum(
                z2s[:], zsq[0:1, :], axis=mybir.AxisListType.X
            )
            lb = fin.tile([1, 2], fp32, tag="lb")
            nc.vector.tensor_copy(lb[:, 0:1], z2s[:])
            nc.vector.tensor_copy(lb[:, 1:2], mxs[:])
            nc.sync.dma_start(lossp[:], lb[:])
            nc.sync.dma_start(idxp[:], ix_all[:])

    nc.compile()
    return nc


_NC_CACHE = {}


def _get_nc(repeat=1, variant="full"):
    key = (repeat, variant)
    if key not in _NC_CACHE:
        _NC_CACHE[key] = _build_graph(repeat, variant)
    return _NC_CACHE[key]


last_exec_time_ns = None
last_profile = None


def prep_in_maps(z, codebooks):
    # token t = b*(H*W) + h*W + w ; zt[n, d, t]
    zt = (
        z.reshape(B, N, DS, H * W)
        .transpose(1, 2, 0, 3)
        .reshape(N, DS, T)
    )
    c2 = (codebooks.astype(np.float64) ** 2).sum(-1).astype(np.float32)  # [N, M]
    cbt = codebooks.transpose(0, 2, 1)  # [N, DS, M]
    cbta = np.concatenate([cbt, (-0.5 * c2)[:, None, :]], axis=1)  # [N, DS+1, M]
    cbta = np.ascontiguousarray(cbta)

    ones_row = np.ones((N, 1, TL), dtype=np.float32)
    in_maps = []
    for i in range(NCORES):
        zs = zt[:, :, i * TL : (i + 1) * TL]  # [N, DS, TL]
        zta = np.ascontiguousarray(np.concatenate([zs, ones_row], axis=1))
        m = {"zta": zta, "cbta": cbta}
        for n in range(N):
            m[f"cbf{n}"] = np.ascontiguousarray(codebooks[n])
        in_maps.append(m)
    return in_maps


def kernel(z, codebooks):
    import os

    from concourse.bass_utils import run_bass_kernel_spmd

    trace = bool(int(os.environ.get("VQ_TRACE", "0")))

    z = np.ascontiguousarray(z, dtype=np.float32)
    codebooks = np.ascontiguousarray(codebooks, dtype=np.float32)

    in_maps = prep_in_maps(z, codebooks)
    nc = _get_nc()
    res = run_bass_kernel_spmd(
        nc, in_maps, core_ids=list(range(NCORES)), trace=trace
    )
    results = res.results
    global last_exec_time_ns, last_profile
    last_exec_time_ns = res.exec_time_ns
    last_profile = getattr(res, "profile_json", None)

    # ---- host-side unshard ----
    out_tok = np.empty((T, N, DS), dtype=np.float32)
    indices = np.empty((T, N), dtype=np.int32)
    z2_sum = 0.0
    mx_sum = 0.0
    for i in range(NCORES):
        r = results[i]
        zqc = np.asarray(r["zq"])  # [N, NT, 128, DS]
        ixp = np.asarray(r["idxp"]).reshape(128, N, NT, 8)
        lp = np.asarray(r["lossp"]).reshape(2)
        z2_sum += float(lp[0])
        mx_sum += float(lp[1])

        sl = slice(i * TL, (i + 1) * TL)
        # token local index = t*128 + p  -> [NT,128] per codebook
        out_tok[sl] = zqc.transpose(1, 2, 0, 3).reshape(TL, N, DS)
        indices[sl] = (
            ixp[:, :, :, 0].transpose(2, 0, 1).reshape(TL, N).astype(np.int32)
        )

    out = (
        out_tok.reshape(B, H * W, D)
        .transpose(0, 2, 1)
        .reshape(B, D, H, W)
    )
    loss = np.float32((z2_sum - 2.0 * mx_sum) / (N * T * DS))
    return out, loss, loss, indices


# revision 33
# speedup vs baseline: 1.4815x; 1.4815x over previous
"""DCVQ quantizer (vq_codebook) on 8 TRN2 NeuronCores.

Strategy (per spec sharding hint): data-parallel over tokens (B*H*W),
codebooks replicated on every core. Per core:
  - distances via TensorE matmuls (float32r, full rate):
    m[t, c] = z_t . c_c - 0.5*||c_c||^2  (the -0.5*c2 folded in via an
    appended ones-row on the stationary side); argmin d2 == argmax m.
  - argmax via DVE max8 + max_index over the [128, 1024] PSUM tile.
  - codes gathered on-device via indirect DMA from the codebook in DRAM.
  - losses: loss_vq == loss_commit == mean(min d2) from sum(z^2)
    (ScalarE square+accum) and sum(max m) (ones-matmul partition
    reduction); per-core partial sums combined on host.

kernel(z, codebooks) takes full inputs, returns
(out[B,D,H,W] f32, loss_vq f32, loss_commit f32, indices[T,N] int32)
matching reference.reference().
"""

import numpy as np


def _rep_range(repeat):
    # repeat the whole per-codebook pipeline (benchmarking aid; repeat=1
    # for normal runs)
    for _ in range(repeat):
        yield from range(N)


# ---- problem constants (hardcoded per harness rules) ----
B, D, H, W = 16, 512, 32, 32
N, M, DS = 8, 1024, 64
NCORES = 8
T = B * H * W                      # 16384 tokens
TL = T // NCORES                   # 2048 tokens per core
NT = TL // 128                     # 16 token tiles of 128
MH = M // 2                        # 512 (half the codebook)
USE_F32R = False                   # f32r flips ~0.07% of argmins (too inexact)


def _build_graph(repeat=1, variant="full"):
    """variant: 'mm' | 'max' | 'maxidx' | 'nogather' | 'full' —
    progressively larger subsets of the pipeline (benchmarking aid)."""
    import concourse.bacc as bacc
    import concourse.bass as bass
    import concourse.mybir as mybir
    from concourse.tile import TileContext

    import os

    lvl = ["mm", "max", "maxidx", "nogather", "full"].index(variant)
    use_f32r = bool(int(os.environ.get("VQ_F32R", "1" if USE_F32R else "0")))

    fp32 = mybir.dt.float32
    u32 = mybir.dt.uint32
    mm_dt = mybir.dt.float32r if use_f32r else fp32

    nc = bacc.Bacc("TRN2", target_bir_lowering=False, debug=False)

    zta = nc.declare_dram_parameter("zta", [N, DS + 1, TL], mm_dt, isOutput=False)
    cbta = nc.declare_dram_parameter("cbta", [N, DS + 1, M], mm_dt, isOutput=False)
    cbf = [
        nc.declare_dram_parameter(f"cbf{n}", [M, DS], fp32, isOutput=False)
        for n in range(N)
    ]
    zq = nc.declare_dram_parameter("zq", [N, NT, 128, DS], fp32, isOutput=True)
    idxp = nc.declare_dram_parameter("idxp", [128, N * NT * 8], u32, isOutput=True)
    lossp = nc.declare_dram_parameter("lossp", [1, 2], fp32, isOutput=True)

    with TileContext(nc) as tc:
        with (
            tc.tile_pool(name="cbp", bufs=2) as cbp,
            tc.tile_pool(name="zp", bufs=2) as zp,
            tc.tile_pool(name="ps", bufs=4, space="PSUM") as psp,
            tc.tile_pool(name="zqp", bufs=4) as zqp,
            tc.tile_pool(name="persist", bufs=1) as pp,
            tc.tile_pool(name="sq", bufs=2) as sqp,
            tc.tile_pool(name="fin", bufs=1) as fin,
        ):
            ix_all = pp.tile([128, N * NT * 8], u32)
            mx_all = pp.tile([128, N * NT * 8], fp32)
            zsq = pp.tile([128, N], fp32)
            nc.vector.memset(zsq[:], 0.0)
            if lvl < 4:  # benchmark variants leave parts unwritten
                nc.vector.memset(ix_all[:], 0)
                nc.vector.memset(mx_all[:], 0.0)

            for n in _rep_range(repeat):
                # DMA into raw staging tiles, then same-dtype DVE copies.
                # Matmuls read only DVE-produced tiles + PSUM guarded by DVE
                # readers, so each (self-loading, 1-wait-limited) matmul
                # carries exactly one semaphore wait (the DVE sem).
                cbt_raw = cbp.tile([DS + 1, M], mm_dt, tag="cbraw")
                nc.sync.dma_start(cbt_raw[:], cbta[n])
                zt_raw = zp.tile([DS + 1, TL], mm_dt, tag="zraw")
                nc.sync.dma_start(zt_raw[:], zta[n])
                cbt = cbp.tile([DS + 1, M], mm_dt, tag="cb")
                nc.vector.tensor_copy(cbt[:], cbt_raw[:])
                zt = zp.tile([DS + 1, TL], mm_dt, tag="z")
                nc.vector.tensor_copy(zt[:], zt_raw[:])

                # sum of z^2 for this codebook slab (loss term)
                junk = sqp.tile([DS, TL], fp32)
                nc.scalar.activation(
                    junk[:],
                    zt_raw[0:DS, :].bitcast(fp32),
                    mybir.ActivationFunctionType.Square,
                    accum_out=zsq[0:DS, n : n + 1],
                )

                for t in range(NT):
                    k = n * NT + t
                    lhs = zt[:, t * 128 : (t + 1) * 128]
                    dm = psp.tile([128, M], fp32)
                    nc.tensor.matmul(
                        dm[:, 0:MH], lhsT=lhs, rhs=cbt[:, 0:MH],
                        start=True, stop=True,
                    )
                    nc.tensor.matmul(
                        dm[:, MH:M], lhsT=lhs, rhs=cbt[:, MH:M],
                        start=True, stop=True,
                    )
                    # row max (top-8) + index of the max, straight off PSUM
                    if lvl >= 1:
                        nc.vector.max(
                            out=mx_all[:, k * 8 : (k + 1) * 8], in_=dm[:]
                        )
                    if lvl >= 2:
                        nc.vector.max_index(
                            out=ix_all[:, k * 8 : (k + 1) * 8],
                            in_max=mx_all[:, k * 8 : (k + 1) * 8],
                            in_values=dm[:],
                        )
                    if lvl >= 4:
                        # gather the winning codes
                        zg = zqp.tile([128, DS], fp32)
                        nc.gpsimd.indirect_dma_start(
                            out=zg[:],
                            out_offset=None,
                            in_=cbf[n][:],
                            in_offset=bass.IndirectOffsetOnAxis(
                                ap=ix_all[:, k * 8 : k * 8 + 1], axis=0
                            ),
                        )
                        nc.sync.dma_start(zq[n, t], zg[:])

            # ---- loss partials (partition reduction on GPSIMD) ----
            from concourse.bass_isa import ReduceOp

            mx0 = mx_all.rearrange("p (k e) -> p k e", e=8)[:, :, 0]
            mxc = fin.tile([128, N * NT], fp32, tag="mxc")
            nc.vector.tensor_copy(mxc[:], mx0)
            nc.gpsimd.partition_all_reduce(
                mxc[:], mxc[:], 128, ReduceOp.add
            )
            mxs = fin.tile([1, 1], fp32, tag="mxs")
            nc.vector.reduce_sum(
                mxs[:], mxc[0:1, :], axis=mybir.AxisListType.X
            )
            nc.gpsimd.partition_all_reduce(zsq[:], zsq[:], 128, ReduceOp.add)
            z2s = fin.tile([1, 1], fp32, tag="z2s")
            nc.vector.reduce_sum(
                z2s[:], zsq[0:1, :], axis=mybir.AxisListType.X
            )
            lb = fin.tile([1, 2], fp32, tag="lb")
            nc.vector.tensor_copy(lb[:, 0:1], z2s[:])
            nc.vector.tensor_copy(lb[:, 1:2], mxs[:])
            nc.sync.dma_start(lossp[:], lb[:])
            nc.sync.dma_start(idxp[:], ix_all[:])

    nc.compile()
    return nc


_NC_CACHE = {}


def _get_nc(repeat=1, variant="full"):
    key = (repeat, variant)
    if key not in _NC_CACHE:
        _NC_CACHE[key] = _build_graph(repeat, variant)
    return _NC_CACHE[key]


last_exec_time_ns = None
last_profile = None


def prep_in_maps(z, codebooks):
    # token t = b*(H*W) + h*W + w ; zt[n, d, t]
    zt = (
        z.reshape(B, N, DS, H * W)
        .transpose(1, 2, 0, 3)
        .reshape(N, DS, T)
    )
    c2 = (codebooks.astype(np.float64) ** 2).sum(-1).astype(np.float32)  # [N, M]
    cbt = codebooks.transpose(0, 2, 1)  # [N, DS, M]
    cbta = np.concatenate([cbt, (-0.5 * c2)[:, None, :]], axis=1)  # [N, DS+1, M]
    cbta = np.ascontiguousarray(cbta)

    ones_row = np.ones((N, 1, TL), dtype=np.float32)
    in_maps = []
    for i in range(NCORES):
        zs = zt[:, :, i * TL : (i + 1) * TL]  # [N, DS, TL]
        zta = np.ascontiguousarray(np.concatenate([zs, ones_row], axis=1))
        m = {"zta": zta, "cbta": cbta}
        for n in range(N):
            m[f"cbf{n}"] = np.ascontiguousarray(codebooks[n])
        in_maps.append(m)
    return in_maps


def kernel(z, codebooks):
    import os

    from concourse.bass_utils import run_bass_kernel_spmd

    trace = bool(int(os.environ.get("VQ_TRACE", "0")))

    z = np.ascontiguousarray(z, dtype=np.float32)
    codebooks = np.ascontiguousarray(codebooks, dtype=np.float32)

    in_maps = prep_in_maps(z, codebooks)
    nc = _get_nc()
    res = run_bass_kernel_spmd(
        nc, in_maps, core_ids=list(range(NCORES)), trace=trace
    )
    results = res.results
    global last_exec_time_ns, last_profile
    last_exec_time_ns = res.exec_time_ns
    last_profile = getattr(res, "profile_json", None)

    # ---- host-side unshard ----
    out_tok = np.empty((T, N, DS), dtype=np.float32)
    indices = np.empty((T, N), dtype=np.int32)
    z2_sum = 0.0
    mx_sum = 0.0
    for i in range(NCORES):
        r = results[i]
        zqc = np.asarray(r["zq"])  # [N, NT, 128, DS]
        ixp = np.asarray(r["idxp"]).reshape(128, N, NT, 8)
        lp = np.asarray(r["lossp"]).reshape(2)
        z2_sum += float(lp[0])
        mx_sum += float(lp[1])

        sl = slice(i * TL, (i + 1) * TL)
        # token local index = t*128 + p  -> [NT,128] per codebook
        out_tok[sl] = zqc.transpose(1, 2, 0, 3).reshape(TL, N, DS)
        indices[sl] = (
            ixp[:, :, :, 0].transpose(2, 0, 1).reshape(TL, N).astype(np.int32)
        )

    out = (
        out_tok.reshape(B, H * W, D)
        .transpose(0, 2, 1)
        .reshape(B, D, H, W)
    )
    loss = np.float32((z2_sum - 2.0 * mx_sum) / (N * T * DS))
    return out, loss, loss, indices


# revision 38
# speedup vs baseline: 1.6926x; 1.1426x over previous
"""DCVQ quantizer (vq_codebook) on 8 TRN2 NeuronCores.

Strategy (per spec sharding hint): data-parallel over tokens (B*H*W),
codebooks replicated on every core. Per core:
  - distances via TensorE matmuls (float32r, full rate):
    m[t, c] = z_t . c_c - 0.5*||c_c||^2  (the -0.5*c2 folded in via an
    appended ones-row on the stationary side); argmin d2 == argmax m.
  - argmax via DVE max8 + max_index over the [128, 1024] PSUM tile.
  - codes gathered on-device via indirect DMA from the codebook in DRAM.
  - losses: loss_vq == loss_commit == mean(min d2) from sum(z^2)
    (ScalarE square+accum) and sum(max m) (ones-matmul partition
    reduction); per-core partial sums combined on host.

kernel(z, codebooks) takes full inputs, returns
(out[B,D,H,W] f32, loss_vq f32, loss_commit f32, indices[T,N] int32)
matching reference.reference().
"""

import numpy as np


def _rep_range(repeat):
    # repeat the whole per-codebook pipeline (benchmarking aid; repeat=1
    # for normal runs)
    for _ in range(repeat):
        yield from range(N)


# ---- problem constants (hardcoded per harness rules) ----
B, D, H, W = 16, 512, 32, 32
N, M, DS = 8, 1024, 64
NCORES = 8
T = B * H * W                      # 16384 tokens
TL = T // NCORES                   # 2048 tokens per core
NT = TL // 128                     # 16 token tiles of 128
MH = M // 2                        # 512 (half the codebook)
MM_MODE = "split"  # f32: exact/slow; f32r: fast, flips 0.07% of argmins;
#                    split: z=hi+lo bf16 decomposition, 3 accumulated bf16
#                    matmuls, ~2^-16 relative error (argmin-exact in practice)


def _build_graph(repeat=1, variant="full"):
    """variant: 'mm' | 'max' | 'maxidx' | 'nogather' | 'full' —
    progressively larger subsets of the pipeline (benchmarking aid)."""
    import concourse.bacc as bacc
    import concourse.bass as bass
    import concourse.mybir as mybir
    from concourse.tile import TileContext

    import os

    lvl = ["mm", "max", "maxidx", "nogather", "full"].index(variant)
    mm_mode = os.environ.get("VQ_MM", MM_MODE)  # f32 | f32r | split

    fp32 = mybir.dt.float32
    bf16 = mybir.dt.bfloat16
    u32 = mybir.dt.uint32
    split = mm_mode == "split"
    mm_dt = mybir.dt.float32r if mm_mode == "f32r" else fp32

    nc = bacc.Bacc("TRN2", target_bir_lowering=False, debug=False)

    if split:
        ztaH = nc.declare_dram_parameter("ztaH", [N, DS + 1, TL], bf16, isOutput=False)
        ztaL = nc.declare_dram_parameter("ztaL", [N, DS + 1, TL], bf16, isOutput=False)
        cbtaH = nc.declare_dram_parameter("cbtaH", [N, DS + 1, M], bf16, isOutput=False)
        cbtaL = nc.declare_dram_parameter("cbtaL", [N, DS + 1, M], bf16, isOutput=False)
    else:
        zta = nc.declare_dram_parameter("zta", [N, DS + 1, TL], mm_dt, isOutput=False)
        cbta = nc.declare_dram_parameter("cbta", [N, DS + 1, M], mm_dt, isOutput=False)
    cbf = [
        nc.declare_dram_parameter(f"cbf{n}", [M, DS], fp32, isOutput=False)
        for n in range(N)
    ]
    zq = nc.declare_dram_parameter("zq", [N, NT, 128, DS], fp32, isOutput=True)
    idxp = nc.declare_dram_parameter("idxp", [128, N * NT * 8], u32, isOutput=True)
    lossp = nc.declare_dram_parameter("lossp", [1, 2], fp32, isOutput=True)

    with TileContext(nc) as tc:
        with (
            tc.tile_pool(name="cbp", bufs=2) as cbp,
            tc.tile_pool(name="zp", bufs=2) as zp,
            tc.tile_pool(name="ps", bufs=4, space="PSUM") as psp,
            tc.tile_pool(name="zqp", bufs=4) as zqp,
            tc.tile_pool(name="persist", bufs=1) as pp,
            tc.tile_pool(name="sq", bufs=2) as sqp,
            tc.tile_pool(name="fin", bufs=1) as fin,
        ):
            ix_all = pp.tile([128, N * NT * 8], u32)
            mx_all = pp.tile([128, N * NT * 8], fp32)
            zsq = pp.tile([128, N], fp32)
            nc.vector.memset(zsq[:], 0.0)
            if lvl < 4:  # benchmark variants leave parts unwritten
                nc.vector.memset(ix_all[:], 0)
                nc.vector.memset(mx_all[:], 0.0)

            for n in _rep_range(repeat):
                if split:
                    cbH = cbp.tile([DS + 1, M], bf16, tag="cbH")
                    nc.sync.dma_start(cbH[:], cbtaH[n])
                    cbL = cbp.tile([DS + 1, M], bf16, tag="cbL")
                    nc.sync.dma_start(cbL[:], cbtaL[n])
                    ztH = zp.tile([DS + 1, TL], bf16, tag="zH")
                    nc.sync.dma_start(ztH[:], ztaH[n])
                    ztL = zp.tile([DS + 1, TL], bf16, tag="zL")
                    nc.sync.dma_start(ztL[:], ztaL[n])

                    # z^2 loss term: zf = zH + zL (GPSIMD), square+accum (ACT)
                    zf = sqp.tile([DS, TL], fp32, tag="zf")
                    nc.gpsimd.tensor_tensor(
                        zf[:], ztH[0:DS, :], ztL[0:DS, :],
                        op=mybir.AluOpType.add,
                    )
                    junk = sqp.tile([DS, TL], fp32, tag="junk")
                    nc.scalar.activation(
                        junk[:],
                        zf[:],
                        mybir.ActivationFunctionType.Square,
                        accum_out=zsq[0:DS, n : n + 1],
                    )
                else:
                    # DMA into raw staging tiles, then same-dtype DVE copies
                    # (keeps every matmul wait on the single DVE semaphore —
                    # self-loading f32/f32r matmuls allow only one wait).
                    cbt_raw = cbp.tile([DS + 1, M], mm_dt, tag="cbraw")
                    nc.sync.dma_start(cbt_raw[:], cbta[n])
                    zt_raw = zp.tile([DS + 1, TL], mm_dt, tag="zraw")
                    nc.sync.dma_start(zt_raw[:], zta[n])
                    cbt = cbp.tile([DS + 1, M], mm_dt, tag="cb")
                    nc.vector.tensor_copy(cbt[:], cbt_raw[:])
                    zt = zp.tile([DS + 1, TL], mm_dt, tag="z")
                    nc.vector.tensor_copy(zt[:], zt_raw[:])

                    # sum of z^2 for this codebook slab (loss term)
                    junk = sqp.tile([DS, TL], fp32, tag="junk")
                    nc.scalar.activation(
                        junk[:],
                        zt_raw[0:DS, :].bitcast(fp32),
                        mybir.ActivationFunctionType.Square,
                        accum_out=zsq[0:DS, n : n + 1],
                    )

                for t in range(NT):
                    k = n * NT + t
                    sl = slice(t * 128, (t + 1) * 128)
                    dm = psp.tile([128, M], fp32)
                    for c0, c1 in ((0, MH), (MH, M)):
                        if split:
                            nc.tensor.matmul(
                                dm[:, c0:c1], lhsT=ztH[:, sl],
                                rhs=cbH[:, c0:c1], start=True, stop=False,
                            )
                            nc.tensor.matmul(
                                dm[:, c0:c1], lhsT=ztH[:, sl],
                                rhs=cbL[:, c0:c1], start=False, stop=False,
                            )
                            nc.tensor.matmul(
                                dm[:, c0:c1], lhsT=ztL[:, sl],
                                rhs=cbH[:, c0:c1], start=False, stop=True,
                            )
                        else:
                            nc.tensor.matmul(
                                dm[:, c0:c1], lhsT=zt[:, sl],
                                rhs=cbt[:, c0:c1], start=True, stop=True,
                            )
                    # row max (top-8) + index of the max, straight off PSUM
                    if lvl >= 1:
                        nc.vector.max(
                            out=mx_all[:, k * 8 : (k + 1) * 8], in_=dm[:]
                        )
                    if lvl >= 2:
                        nc.vector.max_index(
                            out=ix_all[:, k * 8 : (k + 1) * 8],
                            in_max=mx_all[:, k * 8 : (k + 1) * 8],
                            in_values=dm[:],
                        )
                    if lvl >= 4:
                        # gather the winning codes
                        zg = zqp.tile([128, DS], fp32)
                        nc.gpsimd.indirect_dma_start(
                            out=zg[:],
                            out_offset=None,
                            in_=cbf[n][:],
                            in_offset=bass.IndirectOffsetOnAxis(
                                ap=ix_all[:, k * 8 : k * 8 + 1], axis=0
                            ),
                        )
                        nc.sync.dma_start(zq[n, t], zg[:])

            # ---- loss partials (partition reduction on GPSIMD) ----
            from concourse.bass_isa import ReduceOp

            mx0 = mx_all.rearrange("p (k e) -> p k e", e=8)[:, :, 0]
            mxc = fin.tile([128, N * NT], fp32, tag="mxc")
            nc.vector.tensor_copy(mxc[:], mx0)
            nc.gpsimd.partition_all_reduce(
                mxc[:], mxc[:], 128, ReduceOp.add
            )
            mxs = fin.tile([1, 1], fp32, tag="mxs")
            nc.vector.reduce_sum(
                mxs[:], mxc[0:1, :], axis=mybir.AxisListType.X
            )
            nc.gpsimd.partition_all_reduce(zsq[:], zsq[:], 128, ReduceOp.add)
            z2s = fin.tile([1, 1], fp32, tag="z2s")
            nc.vector.reduce_sum(
                z2s[:], zsq[0:1, :], axis=mybir.AxisListType.X
            )
            lb = fin.tile([1, 2], fp32, tag="lb")
            nc.vector.tensor_copy(lb[:, 0:1], z2s[:])
            nc.vector.tensor_copy(lb[:, 1:2], mxs[:])
            nc.sync.dma_start(lossp[:], lb[:])
            nc.sync.dma_start(idxp[:], ix_all[:])

    nc.compile()
    return nc


_NC_CACHE = {}


def _get_nc(repeat=1, variant="full"):
    key = (repeat, variant)
    if key not in _NC_CACHE:
        _NC_CACHE[key] = _build_graph(repeat, variant)
    return _NC_CACHE[key]


last_exec_time_ns = None
last_profile = None


def prep_in_maps(z, codebooks):
    import os

    import ml_dtypes

    mm_mode = os.environ.get("VQ_MM", MM_MODE)
    split = mm_mode == "split"
    bf16 = ml_dtypes.bfloat16

    # token t = b*(H*W) + h*W + w ; zt[n, d, t]
    zt = (
        z.reshape(B, N, DS, H * W)
        .transpose(1, 2, 0, 3)
        .reshape(N, DS, T)
    )
    c2 = (codebooks.astype(np.float64) ** 2).sum(-1).astype(np.float32)  # [N, M]
    cbt = codebooks.transpose(0, 2, 1)  # [N, DS, M]
    cbta = np.concatenate([cbt, (-0.5 * c2)[:, None, :]], axis=1)  # [N, DS+1, M]
    cbta = np.ascontiguousarray(cbta)
    if split:
        cbtaH = cbta.astype(bf16)
        cbtaL = (cbta - cbtaH.astype(np.float32)).astype(bf16)

    ones_row = np.ones((N, 1, TL), dtype=np.float32)
    in_maps = []
    for i in range(NCORES):
        zs = zt[:, :, i * TL : (i + 1) * TL]  # [N, DS, TL]
        zta = np.ascontiguousarray(np.concatenate([zs, ones_row], axis=1))
        if split:
            ztaH = zta.astype(bf16)
            ztaL = (zta - ztaH.astype(np.float32)).astype(bf16)
            m = {
                "ztaH": ztaH,
                "ztaL": ztaL,
                "cbtaH": cbtaH,
                "cbtaL": cbtaL,
            }
        else:
            m = {"zta": zta, "cbta": cbta}
        for n in range(N):
            m[f"cbf{n}"] = np.ascontiguousarray(codebooks[n])
        in_maps.append(m)
    return in_maps


def kernel(z, codebooks):
    import os

    from concourse.bass_utils import run_bass_kernel_spmd

    trace = bool(int(os.environ.get("VQ_TRACE", "0")))

    z = np.ascontiguousarray(z, dtype=np.float32)
    codebooks = np.ascontiguousarray(codebooks, dtype=np.float32)

    in_maps = prep_in_maps(z, codebooks)
    nc = _get_nc()
    res = run_bass_kernel_spmd(
        nc, in_maps, core_ids=list(range(NCORES)), trace=trace
    )
    results = res.results
    global last_exec_time_ns, last_profile
    last_exec_time_ns = res.exec_time_ns
    last_profile = getattr(res, "profile_json", None)

    # ---- host-side unshard ----
    out_tok = np.empty((T, N, DS), dtype=np.float32)
    indices = np.empty((T, N), dtype=np.int32)
    z2_sum = 0.0
    mx_sum = 0.0
    for i in range(NCORES):
        r = results[i]
        zqc = np.asarray(r["zq"])  # [N, NT, 128, DS]
        ixp = np.asarray(r["idxp"]).reshape(128, N, NT, 8)
        lp = np.asarray(r["lossp"]).reshape(2)
        z2_sum += float(lp[0])
        mx_sum += float(lp[1])

        sl = slice(i * TL, (i + 1) * TL)
        # token local index = t*128 + p  -> [NT,128] per codebook
        out_tok[sl] = zqc.transpose(1, 2, 0, 3).reshape(TL, N, DS)
        indices[sl] = (
            ixp[:, :, :, 0].transpose(2, 0, 1).reshape(TL, N).astype(np.int32)
        )

    out = (
        out_tok.reshape(B, H * W, D)
        .transpose(0, 2, 1)
        .reshape(B, D, H, W)
    )
    loss = np.float32((z2_sum - 2.0 * mx_sum) / (N * T * DS))
    return out, loss, loss, indices
